# revision 15
# baseline (speedup 1.0000x reference)
"""Trainium2 Bass kernel for nn_AttentionModule (channel self-attention).

Reference computation (per batch sample b, with x: [C=512, N=4096]):
    q   = w1 @ x + b1                     # [64, 4096]
    att = softmax(q @ q.T, axis=-1)       # [64, 64]
    out = att @ q                         # [64, 4096]
    y   = w2 @ out + b2 + x               # [512, 4096]

Sharding: data-parallel over batch. B=16 samples, 8 cores, 2 samples/core.
Small weights (w1,b1,w2,b2) replicated to every core.

Per-core design (v4).  The kernel is HBM-bound (16.8 MB in + 16.8 MB out
per core, ~91 us at sustained rate) and the PE is power-throttled to
~1.2 GHz whenever the DMA runs hot, so the whole point is minimum PE
work hidden under a never-idle DMA stream:

  - x is loaded STRAIGHT TO BF16 by casting SWDGE (gpsimd) DMAs — the
    only engine that can convert dtypes in-flight.  HBM read traffic is
    unchanged (fp32 source); SBUF holds only the bf16 copy, which both
    the q-matmul and the residual add consume (residual in bf16 costs
    ~2e-3 rel err vs the 2e-2 budget).
  - Every matmul is bf16 (1 PE cycle/row).
  - q.T for the Gram att = q @ q.T comes from ONE XBAR transpose DMA per
    1024-col row on the otherwise idle sync HWDGE ring — zero PE/DVE
    time.  Gram matmuls for row j are emitted after the q-matmuls of
    row j+1 so they never head-of-line-block the PE.
  - out = att @ q is never materialized: y = w2@(att@q) + b2 + x
    = maug @ q_aug + x with maug = [(w2T.T @ att).T ; b2] computed by a
    single 512-row matmul (stationary att), and q_aug = [q ; ones row].
  - step5 runs 1024-wide moving ([128,1024] PSUM tiles); the residual
    add rides the DVE evacuation (PSUM + bf16 x -> fp32 fin) and stores
    issue on the ACT HWDGE ring so they drain as computed.
"""

import os
import sys
from contextlib import ExitStack

import numpy as np

for _p in ("/opt/trn_rl_repo", "/root/.axon_site/_ro/trn_rl_repo"):
    if os.path.isdir(_p) and _p not in sys.path:
        sys.path.append(_p)

import concourse.bass as bass  # noqa: E402
import concourse.tile as tile  # noqa: E402
from concourse import bacc, mybir  # noqa: E402
from concourse.bass_utils import run_bass_kernel_spmd  # noqa: E402
from concourse.masks import make_identity  # noqa: E402

F32 = mybir.dt.float32
BF16 = mybir.dt.bfloat16
AF = mybir.ActivationFunctionType
ALU = mybir.AluOpType
AX = mybir.AxisListType

B, C, CR = 16, 512, 64
W, H = 64, 64
N = W * H  # 4096
NCORES = 8
BPC = B // NCORES  # samples per core
KC = C // 128  # 4 k-chunks of x / o-chunks of output
NF = 512  # moving-dim tile for the q matmuls
NN = N // NF  # 8 n-chunks
NT = N // 128  # 32 gram blocks per sample
LF = 1024  # DMA piece width (load, store, step5 moving width)
NL = N // LF  # 4 pieces per row
TPR = LF // 128  # transpose blocks per piece row (8)
BPR = LF // NF  # q-matmul n-blocks per piece row (2)


def _build_nc():
    nc = bacc.Bacc(
        "TRN2",
        target_bir_lowering=False,
        debug=False,
        enable_asserts=True,
        num_devices=NCORES,
    )
    x_d = nc.dram_tensor("x", [BPC, C, N], F32, kind="ExternalInput").ap()
    w1_d = nc.dram_tensor("w1", [CR, C], F32, kind="ExternalInput").ap()
    b1_d = nc.dram_tensor("b1", [CR], F32, kind="ExternalInput").ap()
    w2_d = nc.dram_tensor("w2", [C, CR], F32, kind="ExternalInput").ap()
    b2_d = nc.dram_tensor("b2", [C], F32, kind="ExternalInput").ap()
    out_d = nc.dram_tensor("out", [BPC, C, N], F32, kind="ExternalOutput").ap()

    with tile.TileContext(nc) as tc, ExitStack() as ctx:
        singles = ctx.enter_context(tc.tile_pool(name="singles", bufs=1))
        xp = ctx.enter_context(tc.tile_pool(name="xp", bufs=2))
        qp = ctx.enter_context(tc.tile_pool(name="qp", bufs=2))
        qtp = ctx.enter_context(tc.tile_pool(name="qtp", bufs=2))
        map_ = ctx.enter_context(tc.tile_pool(name="maug", bufs=2))
        fin = ctx.enter_context(tc.tile_pool(name="fin", bufs=6))
        small = ctx.enter_context(tc.tile_pool(name="small", bufs=2))
        ps_mm = ctx.enter_context(tc.tile_pool(name="ps_mm", bufs=3, space="PSUM"))
        ps_att = ctx.enter_context(tc.tile_pool(name="ps_att", bufs=1, space="PSUM"))
        ps_o = ctx.enter_context(tc.tile_pool(name="ps_o", bufs=4, space="PSUM"))

        # ---------- tiny gpsimd work FIRST (the gpsimd queue is FIFO and the
        # x cast-loads will occupy it for ~50 us; anything emitted after them
        # would stall every consumer) ----------
        ident = singles.tile([128, 128], F32, tag="ident")
        make_identity(nc, ident)
        b2row = singles.tile([1, C], BF16, tag="b2row")
        nc.gpsimd.dma_start(
            out=b2row, in_=b2_d.rearrange("(one c) -> one c", one=1)
        )
        qaugs = []
        for s in range(BPC):
            qaug = qp.tile([CR + 1, N], BF16, tag="q", name=f"q{s}")
            nc.gpsimd.memset(qaug[CR : CR + 1, :], 1.0)
            qaugs.append(qaug)

        # ---------- x loads: casting SWDGE DMAs (fp32 HBM -> bf16 SBUF) ----
        xts = []
        for s in range(BPC):
            xt = [
                xp.tile([128, N], BF16, tag=f"x{k}", name=f"x{s}_{k}")
                for k in range(KC)
            ]
            for j in range(NL):
                lsl = bass.ts(j, LF)
                for k in range(KC):
                    nc.gpsimd.dma_start(
                        out=xt[k][:, lsl],
                        in_=x_d[s, k * 128 : (k + 1) * 128, lsl],
                    )
            xts.append(xt)

        # ---------- weight loads on the ACT ring ----------
        w1_sb = singles.tile([CR, C], F32, tag="w1")  # [64, 512] natural
        nc.scalar.dma_start(out=w1_sb, in_=w1_d)
        b1_sb = singles.tile([CR, 1], F32, tag="b1")
        nc.scalar.dma_start(out=b1_sb, in_=b1_d.rearrange("(c one) -> c one", one=1))
        w2cs = []
        for oc in range(KC):
            w2c = small.tile([128, CR], F32, tag="w2chunk", name=f"w2c{oc}")
            nc.scalar.dma_start(out=w2c, in_=w2_d[oc * 128 : (oc + 1) * 128, :])
            w2cs.append(w2c)
        # ---------- weight prep (PE transposes via the att psum ring) -----
        # w1T: [512, 64] as [128, 4, 64] bf16 (chunk k = w1[:, 128k:+128].T)
        w1T = singles.tile([128, KC, CR], BF16, tag="w1T")

        def prep_w1():
            for k in range(KC):
                ptp = ps_att.tile([128, CR], F32, tag="att", name=f"w1tp{k}")
                nc.tensor.transpose(
                    ptp, w1_sb[:, k * 128 : (k + 1) * 128], ident[0:CR, 0:CR]
                )
                nc.vector.tensor_copy(w1T[:, k, :], ptp)

        # w2T: [64, 512] bf16 (row j = w2[:, j].T)
        w2T = singles.tile([CR, C], BF16, tag="w2T")

        def prep_w2():
            for oc in range(KC):
                ptp = ps_att.tile([CR, 128], F32, tag="att", name=f"w2tp{oc}")
                nc.tensor.transpose(ptp, w2cs[oc], ident)
                nc.vector.tensor_copy(w2T[:, oc * 128 : (oc + 1) * 128], ptp)

        # ---------- per-sample phases ----------
        state = {}

        def begin_sample(s):
            state[s] = {
                "qaug": qaugs[s],
                "qT": qtp.tile([128, NT, CR], BF16, tag="qT", name=f"qT{s}"),
                "patt": ps_att.tile([CR, CR], F32, tag="att", name=f"att{s}"),
                "maug": map_.tile([CR + 1, C], BF16, tag="maug", name=f"maug{s}"),
            }

        def stream_row(s, j):
            """q matmuls + ACT evac + one transpose DMA for piece row j."""
            st = state[s]
            qaug, qT = st["qaug"], st["qT"]
            xt = xts[s]
            for h in range(BPR):
                n = j * BPR + h
                nsl = bass.ts(n, NF)
                pq = ps_mm.tile([CR, NF], F32, tag="mm", name=f"pq{s}_{n}")
                for k in range(KC):
                    nc.tensor.matmul(
                        pq, w1T[:, k, :], xt[k][:, nsl],
                        start=(k == 0), stop=(k == KC - 1),
                    )
                nc.scalar.activation(
                    qaug[0:CR, nsl], pq, AF.Identity, bias=b1_sb, scale=1.0
                )
            # qT[:, 8j:8j+8, :] <- transpose of qaug[0:64, 1024j:+1024]
            nc.sync.dma_start(
                out=qT[:, j * TPR : (j + 1) * TPR, :],
                in_=qaug[0:CR, bass.ts(j, LF)],
                transpose=True,
            )

        def gram_row(s, j):
            """att-Gram matmuls for the transpose blocks of piece row j."""
            st = state[s]
            qT, patt = st["qT"], st["patt"]
            for t_i in range(TPR * j, TPR * (j + 1)):
                qTs = qT[:, t_i, :]
                nc.tensor.matmul(
                    patt, qTs, qTs, start=(t_i == 0), stop=(t_i == NT - 1)
                )

        def softmax_maug(s):
            st = state[s]
            patt, maug = st["patt"], st["maug"]
            negm = small.tile([CR, 1], F32, tag="negm", name=f"negm{s}")
            nc.vector.tensor_reduce(
                out=negm, in_=patt, axis=AX.X, op=ALU.max, negate=True
            )
            shifted = small.tile([CR, CR], F32, tag="shifted", name=f"shifted{s}")
            nc.vector.tensor_scalar(
                out=shifted, in0=patt, scalar1=negm, scalar2=-80.0,
                op0=ALU.add, op1=ALU.max,
            )
            atte = small.tile([CR, CR], F32, tag="atte", name=f"atte{s}")
            ssum = small.tile([CR, 1], F32, tag="ssum", name=f"ssum{s}")
            nc.scalar.activation(
                atte, shifted, AF.Exp, bias=0.0, scale=1.0, accum_out=ssum
            )
            rsum = small.tile([CR, 1], F32, tag="rsum", name=f"rsum{s}")
            nc.vector.reciprocal(rsum, ssum)
            attn = small.tile([CR, CR], BF16, tag="attn", name=f"attn{s}")
            nc.vector.tensor_scalar_mul(attn, atte, rsum)
            # maug rows 0..63 = (att.T @ w2T) = (w2 @ att).T  in one matmul
            pmT = ps_mm.tile([CR, C], F32, tag="mm", name=f"pmT{s}")
            nc.tensor.matmul(pmT, attn, w2T, start=True, stop=True)
            nc.scalar.copy(maug[0:CR, :], pmT)
            nc.vector.tensor_copy(maug[CR : CR + 1, :], b2row)

        def step5_chunk(s, oc):
            """y[oc] = maug[oc] @ q_aug + x[oc] into fin tiles (no stores)."""
            st = state[s]
            qaug, maug = st["qaug"], st["maug"]
            xt = xts[s]
            osl = slice(oc * 128, (oc + 1) * 128)
            fins = []
            for half in range(NL):
                f = fin.tile([128, LF], F32, tag="fin", name=f"fin{s}_{oc}_{half}")
                for sub in range(BPR):
                    n = half * BPR + sub
                    nsl = bass.ts(n, NF)
                    p5 = ps_o.tile([128, NF], F32, tag="o5", name=f"p5{s}_{oc}_{n}")
                    nc.tensor.matmul(
                        p5, maug[:, osl], qaug[:, nsl], start=True, stop=True
                    )
                    nc.vector.tensor_add(f[:, bass.ts(sub, NF)], p5, xt[oc][:, nsl])
                fins.append((f, half))
            return fins

        def issue_stores(s, oc, fins):
            osl = slice(oc * 128, (oc + 1) * 128)
            for f, half in fins:
                nc.scalar.dma_start(out=out_d[s, osl, bass.ts(half, LF)], in_=f)

        # ---- sample 0 stream (w2 prep slotted into row-0 PE slack) ----
        prep_w1()
        begin_sample(0)
        stream_row(0, 0)
        prep_w2()
        for j in range(1, NL):
            stream_row(0, j)
            gram_row(0, j - 1)
        gram_row(0, NL - 1)
        softmax_maug(0)
        # ---- interleave: s0 step5 with s1 stream ----
        begin_sample(1)
        for i in range(KC):
            fins = step5_chunk(0, i)
            stream_row(1, i)
            if i > 0:
                gram_row(1, i - 1)
            issue_stores(0, i, fins)
        gram_row(1, NL - 1)
        softmax_maug(1)
        for i in range(KC):
            fins = step5_chunk(1, i)
            issue_stores(1, i, fins)

    nc.compile()
    return nc


_NC_CACHE = None


def _get_nc():
    global _NC_CACHE
    if _NC_CACHE is None:
        _NC_CACHE = _build_nc()
    return _NC_CACHE


def _as_f32(a):
    return np.ascontiguousarray(np.asarray(a, dtype=np.float32))


def run(inputs, trace=False):
    """Run on all 8 cores; returns (full output [B,C,W,H], BassKernelResults)."""
    nc = _get_nc()
    x = _as_f32(inputs["x"]).reshape(B, C, N)
    w1 = _as_f32(inputs["w1"])
    b1 = _as_f32(inputs["b1"])
    w2 = _as_f32(inputs["w2"])
    b2 = _as_f32(inputs["b2"])
    in_maps = [
        {
            "x": x[c * BPC : (c + 1) * BPC],
            "w1": w1,
            "b1": b1,
            "w2": w2,
            "b2": b2,
        }
        for c in range(NCORES)
    ]
    res = run_bass_kernel_spmd(nc, in_maps, list(range(NCORES)), trace=trace)
    out = np.concatenate([res.results[c]["out"] for c in range(NCORES)], axis=0)
    return out.reshape(B, C, W, H).astype(np.float32, copy=False), res


def kernel(**inputs):
    out, _ = run(inputs)
    return out
